# revision 18
# baseline (speedup 1.0000x reference)
"""Trainium2 Bass kernel for nn_DenseGraphConvNodeToEdge.

out[b,i,j,o] = y_cols[b,j,o] + y_rows[b,i,o] + y_sum[b,o] + bias[o]
  with y_cols = x @ W0.T, y_rows = x @ W1.T, y_sum = x.sum(1) @ W2.T

Strategy: output is [4,1024,1024,64] = pure memory-regime broadcast
materialization. Shard the row dim i across 8 cores (128 rows/core).

Precision/bandwidth trade (correctness gate is rel_err < 2e-2 of max):
  * output is written as bf16 (rel ~2^-9 ≈ 2e-3) -> 64 MiB/core of HBM
    writes instead of 128 MiB.
  * ALL GEMM operands are bf16, cast on the host. Measured on hw:
    fp32-family matmuls (fp32 LOW_HIGH / fp32r HIGH) carry a ~330ns
    LDWEIGHTS per matmul; bf16 weight loads are 146ns and fully
    pipelined. bf16 streams 1 col/cycle at the 1.2GHz effective PE
    clock of this LNC config -> 512-col matmul every ~427ns; the 512
    main matmuls/core are a ~219us PE floor, which is the wall.
  * main GEMM: K=65, rows 0..63 = x (bf16), row 64 = ones. rhs rows
    0..63 = W1rep (W1.T tiled 16x along free: 8192 = 128 j x 64 o),
    row 64 = base, where base[b,j,o] = y_cols + y_sum + bias is
    precomputed on-chip by small bf16 GEMMs (s2 row in exact fp32) and
    flattened into row 64 via SBUF->SBUF DMA on the gpsimd ring.

Startup: W1rep arrives pre-tiled from DRAM straight into the 3 rotating
rhs buffers (one DMA each on separate rings) — no on-chip fill/round
chain. Tail: the last jblk's output goes out as 8 x 256KiB DMAs so the
drain overlaps the final matmuls.

Per jblk (128 j): 16 matmuls [128 i, 512] -> 8 PSUM tiles [128,1024],
copied f32->bf16 to an SBUF staging tile (DVE/ACT alternating), one
2 MiB output DMA per jblk alternating sync/scalar HWDGE rings.
"""

import numpy as np

B, N, C = 4, 1024, 64
N_CORES = 8
R = N // N_CORES  # 128 rows per core

_CACHE = {}


def _build():
    import concourse.tile as tile
    from concourse import bacc, mybir

    f32 = mybir.dt.float32
    bf16 = mybir.dt.bfloat16

    nc = bacc.Bacc("TRN2", target_bir_lowering=False, debug=False,
                   num_devices=N_CORES)

    xt1b = nc.dram_tensor("xt1b", [B, C + 1, N], bf16, kind="ExternalInput").ap()
    xrt1b = nc.dram_tensor("xrt1b", [B, C + 1, R], bf16, kind="ExternalInput").ap()
    w1w = nc.dram_tensor("w1w", [C, 8192], bf16, kind="ExternalInput").ap()
    w0tb = nc.dram_tensor("w0tb", [C, C], bf16, kind="ExternalInput").ap()
    w2t = nc.dram_tensor("w2t", [C, C], f32, kind="ExternalInput").ap()
    bias_row = nc.dram_tensor("bias_row", [1, C], f32, kind="ExternalInput").ap()
    out_s = nc.dram_tensor("out_s", [B, R, N, C], bf16, kind="ExternalOutput").ap()

    with tile.TileContext(nc) as tc:
        with (
            tc.tile_pool(name="const", bufs=1) as const_pool,
            tc.tile_pool(name="rhs", bufs=1) as rhs_pool,
            tc.tile_pool(name="base", bufs=8) as base_pool,
            tc.tile_pool(name="stage", bufs=3) as stage_pool,
            tc.tile_pool(name="psm", bufs=3, space="PSUM") as psum_main,
            tc.tile_pool(name="pss", bufs=2, space="PSUM") as psum_small,
        ):
            # ---- persistent SBUF state ----
            xt1_bf = const_pool.tile([C + 1, B * N], bf16, tag="xt1b")
            lhsT_sb = const_pool.tile([C + 1, B * R], bf16, tag="lhsT")
            rhs_base = const_pool.tile([C + 1, C], bf16, tag="rhsb")
            w2t_sb = const_pool.tile([C, C], f32, tag="w2t")
            bias_sb = const_pool.tile([1, C], f32, tag="bias")
            xsum_sb = const_pool.tile([C, 1], f32, tag="xsum")
            rhs_bufs = [rhs_pool.tile([C + 1, 8192], bf16, tag=f"rhs{k}",
                                      name=f"rhs{k}")
                        for k in range(3)]

            # ---- input DMAs, spread across the three DGE rings ----
            # Everything at startup hangs off xt1_bf[b0] (xsum -> s2 ->
            # bases -> flattens -> mains): it leads the sync ring, followed
            # by rhs buf0 (needed by the first main matmul). buf1/buf2 go
            # on the scalar ring. gpsimd carries only the flattens.
            nc.sync.dma_start(xt1_bf[:, 0:N], xt1b[0])
            nc.sync.dma_start(w2t_sb[:], w2t[:, :])
            nc.sync.dma_start(bias_sb[:], bias_row[:, :])
            nc.sync.dma_start(lhsT_sb[:, 0:R], xrt1b[0])
            nc.sync.dma_start(rhs_bufs[0][:C, :], w1w[:, :])
            nc.scalar.dma_start(rhs_base[0:C, :], w0tb[:, :])
            nc.scalar.dma_start(rhs_bufs[1][:C, :], w1w[:, :])
            nc.scalar.dma_start(rhs_bufs[2][:C, :], w1w[:, :])
            for b in range(1, B):
                nc.sync.dma_start(lhsT_sb[:, b * R:(b + 1) * R], xrt1b[b])
                nc.scalar.dma_start(xt1_bf[:, b * N:(b + 1) * N], xt1b[b])

            copy_idx = 0  # alternate DVE / ACT for PSUM->SBUF copies
            for b in range(B):
                # xsum[c] = sum_j x[b,j,c] (bf16 in, f32 accumulate)
                nc.vector.reduce_sum(
                    xsum_sb[:], xt1_bf[0:C, b * N:(b + 1) * N],
                    axis=mybir.AxisListType.X)
                # s2_row[o] = sum_c xsum[c] * W2[o,c] + bias[o] (exact fp32)
                ps_s2 = psum_small.tile([1, C], f32, tag="pss")
                nc.tensor.matmul(ps_s2[:], xsum_sb[:], w2t_sb[:],
                                 start=True, stop=True)
                nc.vector.tensor_add(rhs_base[C:C + 1, :], ps_s2[:], bias_sb[:])

                # precompute all 8 base tiles for this b up front so the
                # per-chunk critical chain is only flatten-DMA -> mm
                base_tiles = []
                for jblk in range(8):
                    # base tile [128 j, 64 o] (bf16 GEMM, f32 accumulate)
                    ps_b = psum_small.tile([128, C], f32, tag="pss")
                    nc.tensor.matmul(
                        ps_b[:],
                        xt1_bf[:, b * N + jblk * 128: b * N + (jblk + 1) * 128],
                        rhs_base[:],
                        start=True, stop=True)
                    base_r = base_pool.tile([128, C], bf16, tag="base",
                                            name=f"base_r_{b}_{jblk}")
                    nc.vector.tensor_copy(base_r[:], ps_b[:])
                    base_tiles.append(base_r)

                lhsT = lhsT_sb[:, b * R:(b + 1) * R]
                for jblk in range(8):
                    # flatten [128 j, 64 o] -> row 64 of the rhs buffer
                    # (gpsimd/SWDGE: don't queue behind 2 MiB output DMAs on
                    # the sync HWDGE FIFO — the matmuls below block on these)
                    rhs = rhs_bufs[(b * 8 + jblk) % 3]
                    nc.gpsimd.dma_start(
                        rhs[C:C + 1, :].rearrange("a (p o) -> a p o", p=128),
                        base_tiles[jblk][:])
                    # main GEMMs: 16 x [128, 512] single-pass bf16 matmuls
                    last = (b == B - 1 and jblk == 7)
                    stage_t = stage_pool.tile([128, 8192], bf16, tag="stage")
                    j0 = jblk * 128
                    for g in range(8):  # psum groups of [128, 1024]
                        ps_m = psum_main.tile([128, 1024], f32, tag="psm")
                        # 512 cols is the hw max per matmul (1 PSUM bank;
                        # walrus rejects bank-crossing outputs)
                        for h in range(2):
                            nc.tensor.matmul(
                                ps_m[:, h * 512:(h + 1) * 512],
                                lhsT,
                                rhs[:, g * 1024 + h * 512:
                                    g * 1024 + (h + 1) * 512],
                                start=True, stop=True)
                        dst = stage_t[:, g * 1024:(g + 1) * 1024]
                        if copy_idx % 2 == 0:
                            nc.vector.tensor_copy(dst, ps_m[:])
                        else:
                            nc.scalar.copy(dst, ps_m[:])
                        copy_idx += 1
                        if last:
                            # drain the final jblk as 8 x 256KiB DMAs so the
                            # writes overlap the last matmuls/copies
                            dma_eng = nc.sync if g % 2 == 0 else nc.scalar
                            dma_eng.dma_start(
                                out_s[b, :, j0 + g * 16:j0 + (g + 1) * 16, :],
                                dst)
                    if not last:
                        # alternate sync/scalar HWDGE rings so consecutive
                        # 2 MiB writes overlap their completion latency
                        dma_eng = nc.sync if (b * 8 + jblk) % 2 == 0 else nc.scalar
                        dma_eng.dma_start(out_s[b, :, j0:j0 + 128, :], stage_t[:])

    nc.compile()
    return nc


def _get_nc():
    if "nc" not in _CACHE:
        _CACHE["nc"] = _build()
    return _CACHE["nc"]


def kernel(x, adj, W0, W1, W2, bias):
    import ml_dtypes
    from concourse.bass_utils import run_bass_kernel_spmd

    bf = ml_dtypes.bfloat16
    x = np.ascontiguousarray(np.asarray(x, dtype=np.float32))
    W0 = np.asarray(W0, dtype=np.float32)
    W1 = np.asarray(W1, dtype=np.float32)
    W2 = np.asarray(W2, dtype=np.float32)
    bias = np.asarray(bias, dtype=np.float32)

    nc = _get_nc()

    ones_n = np.ones((B, 1, N), dtype=np.float32)
    xt1b = np.ascontiguousarray(np.concatenate(
        [x.transpose(0, 2, 1), ones_n], axis=1).astype(bf))
    w1w = np.ascontiguousarray(np.tile(W1.T.astype(bf), (1, 128)))
    w0tb = np.ascontiguousarray(W0.T.astype(bf))
    w2t = np.ascontiguousarray(W2.T)
    bias_row = np.ascontiguousarray(bias.T)

    in_maps = []
    ones_r = np.ones((B, 1, R), dtype=np.float32)
    for c in range(N_CORES):
        xr = x[:, c * R:(c + 1) * R, :]
        xrt1b = np.ascontiguousarray(np.concatenate(
            [xr.transpose(0, 2, 1), ones_r], axis=1).astype(bf))
        in_maps.append({
            "xt1b": xt1b, "xrt1b": xrt1b, "w1w": w1w,
            "w0tb": w0tb, "w2t": w2t, "bias_row": bias_row,
        })

    global _last_in_maps
    _last_in_maps = in_maps
    res = run_bass_kernel_spmd(nc, in_maps, list(range(N_CORES)))

    out = np.empty((B, N, N, C), dtype=np.float32)
    for c in range(N_CORES):
        out[:, c * R:(c + 1) * R] = np.asarray(
            res.results[c]["out_s"]).astype(np.float32)
    return out


# revision 23
# speedup vs baseline: 1.0026x; 1.0026x over previous
"""Trainium2 Bass kernel for nn_DenseGraphConvNodeToEdge.

out[b,i,j,o] = y_cols[b,j,o] + y_rows[b,i,o] + y_sum[b,o] + bias[o]
  with y_cols = x @ W0.T, y_rows = x @ W1.T, y_sum = x.sum(1) @ W2.T

Strategy: output is [4,1024,1024,64] = pure memory-regime broadcast
materialization. Shard the row dim i across 8 cores (128 rows/core).

Precision/bandwidth trade (correctness gate is rel_err < 2e-2 of max):
  * output is written as bf16 (rel ~2^-9 ≈ 2e-3) -> 64 MiB/core of HBM
    writes instead of 128 MiB.
  * ALL GEMM operands are bf16, cast on the host. Measured on hw:
    fp32-family matmuls (fp32 LOW_HIGH / fp32r HIGH) carry a ~330ns
    LDWEIGHTS per matmul; bf16 weight loads are 146ns and fully
    pipelined. bf16 streams 1 col/cycle at the 1.2GHz effective PE
    clock of this LNC config -> 512-col matmul every ~427ns; the 512
    main matmuls/core are a ~219us PE floor, which is the wall.
  * main GEMM: K=65, rows 0..63 = x (bf16), row 64 = ones. rhs rows
    0..63 = W1rep (W1.T tiled 16x along free: 8192 = 128 j x 64 o),
    row 64 = base, where base[b,j,o] = y_cols + y_sum + bias is
    precomputed on-chip by small bf16 GEMMs (s2 row in exact fp32) and
    flattened into row 64 via SBUF->SBUF DMA on the gpsimd ring.

Startup: W1rep arrives pre-tiled from DRAM straight into the 3 rotating
rhs buffers (one DMA each on separate rings) — no on-chip fill/round
chain. Tail: the last jblk's output goes out as 8 x 256KiB DMAs so the
drain overlaps the final matmuls.

Per jblk (128 j): 16 matmuls [128 i, 512] -> 8 PSUM tiles [128,1024],
copied f32->bf16 to an SBUF staging tile (DVE/ACT alternating), one
2 MiB output DMA per jblk alternating sync/scalar HWDGE rings.
"""

import numpy as np

B, N, C = 4, 1024, 64
N_CORES = 8
R = N // N_CORES  # 128 rows per core

_CACHE = {}


def _build():
    import concourse.tile as tile
    from concourse import bacc, mybir

    f32 = mybir.dt.float32
    bf16 = mybir.dt.bfloat16

    nc = bacc.Bacc("TRN2", target_bir_lowering=False, debug=False,
                   num_devices=N_CORES)

    # b-major-inside-free layouts: one wide DMA per tensor (65 big packets
    # instead of 4x65 small ones — startup DMA is packet-latency-bound)
    xt1b = nc.dram_tensor("xt1b", [C + 1, B * N], bf16, kind="ExternalInput").ap()
    xrt1b = nc.dram_tensor("xrt1b", [C + 1, B * R], bf16, kind="ExternalInput").ap()
    w1w = nc.dram_tensor("w1w", [C, 8192], bf16, kind="ExternalInput").ap()
    w0tb = nc.dram_tensor("w0tb", [C, C], bf16, kind="ExternalInput").ap()
    w2t = nc.dram_tensor("w2t", [C, C], f32, kind="ExternalInput").ap()
    bias_row = nc.dram_tensor("bias_row", [1, C], f32, kind="ExternalInput").ap()
    out_s = nc.dram_tensor("out_s", [B, R, N, C], bf16, kind="ExternalOutput").ap()

    with tile.TileContext(nc) as tc:
        with (
            tc.tile_pool(name="const", bufs=1) as const_pool,
            tc.tile_pool(name="rhs", bufs=1) as rhs_pool,
            tc.tile_pool(name="base", bufs=8) as base_pool,
            tc.tile_pool(name="stage", bufs=3) as stage_pool,
            tc.tile_pool(name="psm", bufs=3, space="PSUM") as psum_main,
            tc.tile_pool(name="pss", bufs=2, space="PSUM") as psum_small,
        ):
            # ---- persistent SBUF state ----
            xt1_bf = const_pool.tile([C + 1, B * N], bf16, tag="xt1b")
            lhsT_sb = const_pool.tile([C + 1, B * R], bf16, tag="lhsT")
            rhs_base = const_pool.tile([C + 1, C], bf16, tag="rhsb")
            w2t_sb = const_pool.tile([C, C], f32, tag="w2t")
            bias_sb = const_pool.tile([1, C], f32, tag="bias")
            xsum_sb = const_pool.tile([C, 1], f32, tag="xsum")
            rhs_bufs = [rhs_pool.tile([C + 1, 8192], bf16, tag=f"rhs{k}",
                                      name=f"rhs{k}")
                        for k in range(3)]

            # ---- input DMAs, spread across the three DGE rings ----
            # Startup critical path is max(buf0 1MiB on sync; xt1b[b0] ->
            # xsum -> s2 -> bases -> flatten-j0 on scalar+gpsimd). Small
            # tensors ride the otherwise-idle gpsimd ring.
            nc.sync.dma_start(rhs_bufs[0][:C, :], w1w[:, :])
            nc.sync.dma_start(xt1_bf[:, N:B * N], xt1b[:, N:B * N])
            nc.scalar.dma_start(xt1_bf[:, 0:N], xt1b[:, 0:N])
            nc.scalar.dma_start(rhs_base[0:C, :], w0tb[:, :])
            nc.scalar.dma_start(rhs_bufs[1][:C, :], w1w[:, :])
            nc.scalar.dma_start(rhs_bufs[2][:C, :], w1w[:, :])
            nc.gpsimd.dma_start(w2t_sb[:], w2t[:, :])
            nc.gpsimd.dma_start(bias_sb[:], bias_row[:, :])
            nc.gpsimd.dma_start(lhsT_sb[:], xrt1b[:, :])

            copy_idx = 0  # alternate DVE / ACT for PSUM->SBUF copies
            for b in range(B):
                # xsum[c] = sum_j x[b,j,c] (bf16 in, f32 accumulate)
                nc.vector.reduce_sum(
                    xsum_sb[:], xt1_bf[0:C, b * N:(b + 1) * N],
                    axis=mybir.AxisListType.X)
                # s2_row[o] = sum_c xsum[c] * W2[o,c] + bias[o] (exact fp32)
                ps_s2 = psum_small.tile([1, C], f32, tag="pss")
                nc.tensor.matmul(ps_s2[:], xsum_sb[:], w2t_sb[:],
                                 start=True, stop=True)
                nc.vector.tensor_add(rhs_base[C:C + 1, :], ps_s2[:], bias_sb[:])

                # precompute all 8 base tiles for this b up front so the
                # per-chunk critical chain is only flatten-DMA -> mm
                base_tiles = []
                for jblk in range(8):
                    # base tile [128 j, 64 o] (bf16 GEMM, f32 accumulate)
                    ps_b = psum_small.tile([128, C], f32, tag="pss")
                    nc.tensor.matmul(
                        ps_b[:],
                        xt1_bf[:, b * N + jblk * 128: b * N + (jblk + 1) * 128],
                        rhs_base[:],
                        start=True, stop=True)
                    base_r = base_pool.tile([128, C], bf16, tag="base",
                                            name=f"base_r_{b}_{jblk}")
                    nc.vector.tensor_copy(base_r[:], ps_b[:])
                    base_tiles.append(base_r)

                lhsT = lhsT_sb[:, b * R:(b + 1) * R]
                for jblk in range(8):
                    # flatten [128 j, 64 o] -> row 64 of the rhs buffer
                    # (gpsimd/SWDGE: don't queue behind 2 MiB output DMAs on
                    # the sync HWDGE FIFO — the matmuls below block on these)
                    rhs = rhs_bufs[(b * 8 + jblk) % 3]
                    nc.gpsimd.dma_start(
                        rhs[C:C + 1, :].rearrange("a (p o) -> a p o", p=128),
                        base_tiles[jblk][:])
                    # main GEMMs: 16 x [128, 512] single-pass bf16 matmuls
                    last = (b == B - 1 and jblk >= 6)
                    stage_t = stage_pool.tile([128, 8192], bf16, tag="stage")
                    j0 = jblk * 128
                    for g in range(8):  # psum groups of [128, 1024]
                        ps_m = psum_main.tile([128, 1024], f32, tag="psm")
                        # 512 cols is the hw max per matmul (1 PSUM bank;
                        # walrus rejects bank-crossing outputs)
                        for h in range(2):
                            nc.tensor.matmul(
                                ps_m[:, h * 512:(h + 1) * 512],
                                lhsT,
                                rhs[:, g * 1024 + h * 512:
                                    g * 1024 + (h + 1) * 512],
                                start=True, stop=True)
                        dst = stage_t[:, g * 1024:(g + 1) * 1024]
                        if copy_idx % 2 == 0:
                            nc.vector.tensor_copy(dst, ps_m[:])
                        else:
                            nc.scalar.copy(dst, ps_m[:])
                        copy_idx += 1
                        if last:
                            # drain the final jblk as 8 x 256KiB DMAs so the
                            # writes overlap the last matmuls/copies
                            dma_eng = nc.sync if g % 2 == 0 else nc.scalar
                            dma_eng.dma_start(
                                out_s[b, :, j0 + g * 16:j0 + (g + 1) * 16, :],
                                dst)
                    if not last:
                        # alternate sync/scalar HWDGE rings so consecutive
                        # 2 MiB writes overlap their completion latency
                        dma_eng = nc.sync if (b * 8 + jblk) % 2 == 0 else nc.scalar
                        dma_eng.dma_start(out_s[b, :, j0:j0 + 128, :], stage_t[:])

    nc.compile()
    return nc


def _get_nc():
    if "nc" not in _CACHE:
        _CACHE["nc"] = _build()
    return _CACHE["nc"]


def kernel(x, adj, W0, W1, W2, bias):
    import ml_dtypes
    from concourse.bass_utils import run_bass_kernel_spmd

    bf = ml_dtypes.bfloat16
    x = np.ascontiguousarray(np.asarray(x, dtype=np.float32))
    W0 = np.asarray(W0, dtype=np.float32)
    W1 = np.asarray(W1, dtype=np.float32)
    W2 = np.asarray(W2, dtype=np.float32)
    bias = np.asarray(bias, dtype=np.float32)

    nc = _get_nc()

    ones_n = np.ones((B, 1, N), dtype=np.float32)
    # [C+1, B*N] bf16: partition-major with all batches in the free dim
    xt1b = np.ascontiguousarray(np.concatenate(
        [x.transpose(0, 2, 1), ones_n], axis=1).transpose(1, 0, 2)
        .reshape(C + 1, B * N).astype(bf))
    w1w = np.ascontiguousarray(np.tile(W1.T.astype(bf), (1, 128)))
    w0tb = np.ascontiguousarray(W0.T.astype(bf))
    w2t = np.ascontiguousarray(W2.T)
    bias_row = np.ascontiguousarray(bias.T)

    in_maps = []
    ones_r = np.ones((B, 1, R), dtype=np.float32)
    for c in range(N_CORES):
        xr = x[:, c * R:(c + 1) * R, :]
        xrt1b = np.ascontiguousarray(np.concatenate(
            [xr.transpose(0, 2, 1), ones_r], axis=1).transpose(1, 0, 2)
            .reshape(C + 1, B * R).astype(bf))
        in_maps.append({
            "xt1b": xt1b, "xrt1b": xrt1b, "w1w": w1w,
            "w0tb": w0tb, "w2t": w2t, "bias_row": bias_row,
        })

    global _last_in_maps
    _last_in_maps = in_maps
    res = run_bass_kernel_spmd(nc, in_maps, list(range(N_CORES)))

    out = np.empty((B, N, N, C), dtype=np.float32)
    for c in range(N_CORES):
        out[:, c * R:(c + 1) * R] = np.asarray(
            res.results[c]["out_s"]).astype(np.float32)
    return out
